# revision 3
# baseline (speedup 1.0000x reference)
"""Cox hazard loss kernel for Trainium2 (8 NeuronCores, data-parallel over batch).

Math (per batch row b, N players):
  T = where(valid, target, -2)                  # -2 fill makes (T_j >= T_i) == risk_set_mask directly
  m = max_j pred[b, j]                          # i-independent logsumexp shift
  e_j = exp(pred_j - m)
  mask_ij = (T_j >= T_i)                        # == (T_j >= T_i) & valid_j (see -2/-1 analysis)
  e_m[i,j] = mask_ij * e_j ;  S_i = sum_j e_m[i,j]
  p_ij = e_m[i,j] / S_i                          # softmax over risk set
  l_ij = log(1 + EPS - p_ij)
  log_den_i = m + log(S_i)
  loss_i = is_elim_i * ((log_den_i - pred_i) - sum_{j in mask} l_ij + l_ii)
  total = sum_{b,i} loss_i * valid_batch_b ; result = total / max(sum_b valid_batch_b, 1)

Per core: 16 batch rows; per row 4 chunks of 128 i's on partitions, 512 j's on free dim.
Big ops per chunk: 1 STT (mask*e + rowsum S), 1 ACT Ln, 1 STT (masked l rowsum).
Broadcast rows to 128 partitions via K=1 matmul on the (otherwise idle) PE.
"""

import os
import sys

import numpy as np

B, N = 128, 512
NCORES = 8
ROWS = B // NCORES  # 16
P = 128
NCHUNK = N // P  # 4
EPS = 1e-7
# Nudge keeps p = e*recip(S) strictly below 1 even if reciprocal rounds up,
# so Ln(1+EPS-p) never sees a non-positive argument (singleton risk sets hit p==1).
NUDGE = 1.0 - 1e-6

_CACHE = {}


def _ensure_paths():
    for p_ in ("/opt/trn_rl_repo", "/root/.axon_site/_ro/trn_rl_repo"):
        if os.path.isdir(p_) and p_ not in sys.path:
            sys.path.append(p_)


def _build_program():
    _ensure_paths()
    import concourse.bacc as bacc
    import concourse.mybir as mybir
    import concourse.tile as tile

    f32 = mybir.dt.float32
    ALU = mybir.AluOpType
    ACTF = mybir.ActivationFunctionType

    nc = bacc.Bacc("TRN2", target_bir_lowering=False, debug=False, num_devices=NCORES)

    PRED = nc.dram_tensor("PRED", (ROWS, N), f32, kind="ExternalInput").ap()
    TJ = nc.dram_tensor("TJ", (ROWS, N), f32, kind="ExternalInput").ap()
    PREDC = nc.dram_tensor("PREDC", (P, ROWS * NCHUNK), f32, kind="ExternalInput").ap()
    TJC = nc.dram_tensor("TJC", (P, ROWS * NCHUNK), f32, kind="ExternalInput").ap()
    ISELC = nc.dram_tensor("ISELC", (P, ROWS * NCHUNK), f32, kind="ExternalInput").ap()
    MNEGB = nc.dram_tensor("MNEGB", (P, ROWS), f32, kind="ExternalInput").ap()
    ACC = nc.dram_tensor("ACC", (P, NCHUNK), f32, kind="ExternalOutput").ap()

    with tile.TileContext(nc) as tc:
        with (
            tc.tile_pool(name="const", bufs=1) as cp,
            tc.tile_pool(name="row", bufs=2) as rp,
            tc.tile_pool(name="big", bufs=3) as bp,
            tc.tile_pool(name="psum", bufs=2, space="PSUM") as pp,
        ):
            ones = cp.tile([1, P], f32)
            nc.vector.memset(ones[:], 1.0)
            b1eps = cp.tile([P, 1], f32)
            nc.vector.memset(b1eps[:], 1.0 + EPS)
            predc = cp.tile([P, ROWS * NCHUNK], f32)
            nc.sync.dma_start(predc[:], PREDC[:])
            tjc = cp.tile([P, ROWS * NCHUNK], f32)
            nc.sync.dma_start(tjc[:], TJC[:])
            iselc = cp.tile([P, ROWS * NCHUNK], f32)
            nc.sync.dma_start(iselc[:], ISELC[:])
            mnegb = cp.tile([P, ROWS], f32)
            nc.sync.dma_start(mnegb[:], MNEGB[:])
            acc = cp.tile([P, NCHUNK], f32)
            nc.vector.memset(acc[:], 0.0)

            for b in range(ROWS):
                mneg_col = mnegb[:, b : b + 1]
                # Row loads at partition 0 (matmul rhs must start at partition 0/32/64).
                pred_row = rp.tile([1, N], f32, tag="pred_row")
                nc.sync.dma_start(pred_row[:], PRED[b : b + 1, :])
                tj_row = rp.tile([1, N], f32, tag="tj_row")
                nc.sync.dma_start(tj_row[:], TJ[b : b + 1, :])
                # Broadcast pred row / T row across 128 partitions via K=1 matmul.
                pbc_ps = pp.tile([P, N], f32, tag="pbc")
                nc.tensor.matmul(pbc_ps[:], ones[:], pred_row[:], start=True, stop=True)
                tjb_ps = pp.tile([P, N], f32, tag="tjb")
                nc.tensor.matmul(tjb_ps[:], ones[:], tj_row[:], start=True, stop=True)
                # ebc = exp(pred_j - m), broadcast; PSUM -> SBUF through the ACT op.
                ebc = rp.tile([P, N], f32, tag="ebc")
                nc.scalar.activation(ebc[:], pbc_ps[:], ACTF.Exp, bias=mneg_col, scale=1.0)

                S4 = rp.tile([P, NCHUNK], f32, tag="S4")
                nrecip4 = rp.tile([P, NCHUNK], f32, tag="nrecip4")
                lsum4 = rp.tile([P, NCHUNK], f32, tag="lsum4")
                e_ms = []
                for c in range(NCHUNK):
                    cc = b * NCHUNK + c
                    e_m = bp.tile([P, N], f32, tag=f"e_m{c}")
                    e_ms.append(e_m)
                    # e_m = (T_j >= T_i) * e_j ; S = rowsum(e_m)
                    nc.vector.scalar_tensor_tensor(
                        out=e_m[:], in0=tjb_ps[:], scalar=tjc[:, cc : cc + 1], in1=ebc[:],
                        op0=ALU.is_ge, op1=ALU.mult, accum_out=S4[:, c : c + 1],
                    )
                nc.vector.reciprocal(nrecip4[:], S4[:])
                nc.vector.tensor_scalar_mul(nrecip4[:], nrecip4[:], -NUDGE)
                for c in range(NCHUNK):
                    e_m = e_ms[c]
                    l = bp.tile([P, N], f32, tag=f"l{c}")
                    # l = Ln(1 + EPS - e_m / S)
                    nc.scalar.activation(
                        l[:], e_m[:], ACTF.Ln, bias=b1eps[:], scale=nrecip4[:, c : c + 1]
                    )
                    scr = bp.tile([P, N], f32, tag=f"scr{c}")
                    # lsum = rowsum over the risk set of l
                    nc.vector.scalar_tensor_tensor(
                        out=scr[:], in0=e_m[:], scalar=0.0, in1=l[:],
                        op0=ALU.is_gt, op1=ALU.mult, accum_out=lsum4[:, c : c + 1],
                    )

                sl = slice(b * NCHUNK, (b + 1) * NCHUNK)
                # Diagonal term: l_ii = Ln(1 + EPS - e_ii / S)
                e_col = rp.tile([P, NCHUNK], f32, tag="e_col")
                nc.scalar.activation(e_col[:], predc[:, sl], ACTF.Exp, bias=mneg_col, scale=1.0)
                pn4 = rp.tile([P, NCHUNK], f32, tag="pn4")
                nc.vector.tensor_mul(pn4[:], e_col[:], nrecip4[:])
                lii4 = rp.tile([P, NCHUNK], f32, tag="lii4")
                nc.scalar.activation(lii4[:], pn4[:], ACTF.Ln, bias=b1eps[:], scale=1.0)
                logS4 = rp.tile([P, NCHUNK], f32, tag="logS4")
                nc.scalar.activation(logS4[:], S4[:], ACTF.Ln, bias=0.0, scale=1.0)
                # d1 = (logS - (-m)) - pred_i  == log_den - pred_i
                d1 = rp.tile([P, NCHUNK], f32, tag="d1")
                nc.vector.scalar_tensor_tensor(
                    out=d1[:], in0=logS4[:], scalar=mneg_col, in1=predc[:, sl],
                    op0=ALU.subtract, op1=ALU.subtract,
                )
                # d2 = l_ii - lsum
                d2 = rp.tile([P, NCHUNK], f32, tag="d2")
                nc.vector.tensor_sub(d2[:], lii4[:], lsum4[:])
                d3 = rp.tile([P, NCHUNK], f32, tag="d3")
                nc.vector.tensor_add(d3[:], d1[:], d2[:])
                c4 = rp.tile([P, NCHUNK], f32, tag="c4")
                nc.vector.tensor_mul(c4[:], d3[:], iselc[:, sl])
                nc.vector.tensor_add(acc[:], acc[:], c4[:])

            nc.sync.dma_start(ACC[:], acc[:])

    nc.compile()
    return nc


def _get_program():
    if "nc" not in _CACHE:
        _CACHE["nc"] = _build_program()
    return _CACHE["nc"]


def _prep_inputs(pred, target, valid_mask):
    pred = np.ascontiguousarray(pred, dtype=np.float32)
    target = np.ascontiguousarray(target, dtype=np.float32)
    valid = np.ascontiguousarray(valid_mask).astype(bool)

    tj = np.where(valid, target, np.float32(-2.0)).astype(np.float32)
    m = pred.max(axis=1)  # (B,)
    tm = np.where(valid, target, np.float32(-1.0)).astype(np.float32)
    bmax = tm.max(axis=1, keepdims=True)
    is_elim = (tm < bmax) & (tm > 0) & valid
    vbm = (valid.sum(axis=1) >= 2).astype(np.float32)  # (B,)
    isel = is_elim.astype(np.float32) * vbm[:, None]
    num_valid = max(float(vbm.sum()), 1.0)

    in_maps = []
    for s in range(NCORES):
        rs = slice(s * ROWS, (s + 1) * ROWS)
        pred_s, tj_s, isel_s, m_s = pred[rs], tj[rs], isel[rs], m[rs]
        # column layouts: C[p, b*NCHUNK + c] = X[b, c*128 + p]
        def colize(x):
            return np.ascontiguousarray(
                x.reshape(ROWS, NCHUNK, P).transpose(2, 0, 1).reshape(P, ROWS * NCHUNK)
            )
        in_maps.append({
            "PRED": pred_s,
            "TJ": tj_s,
            "PREDC": colize(pred_s),
            "TJC": colize(tj_s),
            "ISELC": colize(isel_s),
            "MNEGB": np.ascontiguousarray(np.broadcast_to(-m_s[None, :], (P, ROWS))),
        })
    return in_maps, num_valid


def _run(inputs, trace=False, **kwargs):
    _ensure_paths()
    from concourse.bass_utils import run_bass_kernel_spmd

    nc = _get_program()
    in_maps, num_valid = _prep_inputs(**inputs)
    res = run_bass_kernel_spmd(nc, in_maps, core_ids=list(range(NCORES)), trace=trace, **kwargs)
    total = np.float32(0.0)
    for r in res.results:
        total += np.float32(r["ACC"].sum(dtype=np.float32))
    out = np.float32(total / np.float32(num_valid))
    return np.asarray(out, dtype=np.float32), res


def kernel(pred, target, valid_mask):
    out, _ = _run({"pred": pred, "target": target, "valid_mask": valid_mask})
    return out


# revision 7
# speedup vs baseline: 1.4443x; 1.4443x over previous
"""Cox hazard loss kernel for Trainium2 (8 NeuronCores, data-parallel over batch).

Math (per batch row b, N players):
  T = where(valid, target, -2)            # -2 fill makes (T_j >= T_i) == risk_set_mask directly
  m = max_j pred[b, j]                    # i-independent logsumexp shift (folded host-side)
  e_j = exp(pred_j - m)
  mask_ij = (T_j >= T_i)
  e_m[i,j] = mask_ij * e_j ;  S_i = sum_j e_m[i,j]
  p_ij = e_m[i,j] / S_i                   # softmax over risk set
  l_ij = log(1 + EPS - p_ij)
  loss_i = is_elim_i * (log(S_i) - (pred_i - m) - sum_{j in mask} l_ij + l_ii)
  total = sum_{b,i} loss_i * valid_batch_b ; result = total / max(sum_b valid_batch_b, 1)

Per core: 16 batch rows; per row 4 chunks of 128 i's on partitions, 512 j's on free dim.
Big ops per chunk: 1 STT (mask*e + rowsum S), 1 ACT Ln, 1 STT (masked l rowsum); all SBUF.
Row broadcasts (T_j row, e row) are done by DMA with a partition-step-0 source AP.
All Exp ops batched up front and per-row epilogues batched at the end so the
scalar engine loads each activation table once (table loads cost ~1.3us each).
"""

import os
import sys

import numpy as np

B, N = 128, 512
NCORES = 8
ROWS = B // NCORES  # 16
P = 128
NCHUNK = N // P  # 4
NC4 = ROWS * NCHUNK  # 64
EPS = 1e-7
# Nudge keeps p = e*recip(S) strictly below 1 even if reciprocal rounds up,
# so Ln(1+EPS-p) never sees a non-positive argument (singleton risk sets hit p==1).
NUDGE = 1.0 - 1e-6

_CACHE = {}


def _ensure_paths():
    for p_ in ("/opt/trn_rl_repo", "/root/.axon_site/_ro/trn_rl_repo"):
        if os.path.isdir(p_) and p_ not in sys.path:
            sys.path.append(p_)


def _build_program():
    _ensure_paths()
    import concourse.bacc as bacc
    import concourse.mybir as mybir
    import concourse.tile as tile

    f32 = mybir.dt.float32
    ALU = mybir.AluOpType
    ACTF = mybir.ActivationFunctionType

    nc = bacc.Bacc("TRN2", target_bir_lowering=False, debug=False, num_devices=NCORES)

    # PREDM: pred - m (rows);  PREDCM: same, column-layout;  TJ: masked target rows;
    # TJC: column-layout;  ISELC: is_elim * valid_batch, column-layout.
    PREDM = nc.dram_tensor("PREDM", (ROWS, N), f32, kind="ExternalInput").ap()
    TJ = nc.dram_tensor("TJ", (ROWS, N), f32, kind="ExternalInput").ap()
    PREDCM = nc.dram_tensor("PREDCM", (P, NC4), f32, kind="ExternalInput").ap()
    TJC = nc.dram_tensor("TJC", (P, NC4), f32, kind="ExternalInput").ap()
    ISELC = nc.dram_tensor("ISELC", (P, NC4), f32, kind="ExternalInput").ap()
    ACC = nc.dram_tensor("ACC", (P, 1), f32, kind="ExternalOutput").ap()

    with tile.TileContext(nc) as tc:
        with (
            tc.tile_pool(name="const", bufs=1) as cp,
            tc.tile_pool(name="row", bufs=3) as rp,
            tc.tile_pool(name="big", bufs=2) as bp,
            tc.tile_pool(name="dram", bufs=1, space="DRAM") as dp,
        ):
            b1eps = cp.tile([P, 1], f32)
            nc.vector.memset(b1eps[:], 1.0 + EPS)
            predcm = cp.tile([P, NC4], f32)
            nc.sync.dma_start(predcm[:], PREDCM[:])
            tjc = cp.tile([P, NC4], f32)
            nc.sync.dma_start(tjc[:], TJC[:])
            iselc = cp.tile([P, NC4], f32)
            nc.sync.dma_start(iselc[:], ISELC[:])
            predm_all = cp.tile([ROWS, N], f32)
            nc.sync.dma_start(predm_all[:], PREDM[:])

            # Batched Exps (one table load)
            e_all = cp.tile([ROWS, N], f32)
            nc.scalar.activation(e_all[:], predm_all[:], ACTF.Exp, bias=0.0, scale=1.0)
            # Bounce e rows through DRAM so they can be partition-broadcast by DMA
            # (SBUF source APs cannot have a zero partition step).
            e_dram = dp.tile([ROWS, N], f32)
            nc.sync.dma_start(e_dram[:], e_all[:])
            e_colall = cp.tile([P, NC4], f32)
            nc.scalar.activation(e_colall[:], predcm[:], ACTF.Exp, bias=0.0, scale=1.0)

            # Full-run accumulators, one column per (row, chunk)
            s_all = cp.tile([P, NC4], f32)
            lsum_all = cp.tile([P, NC4], f32)
            pn_all = cp.tile([P, NC4], f32)

            for b in range(ROWS):
                sl = slice(b * NCHUNK, (b + 1) * NCHUNK)
                # Broadcast T row (from DRAM) and e row (from SBUF) across partitions.
                tjb = rp.tile([P, N], f32, tag="tjb")
                nc.sync.dma_start(tjb[:], TJ[b : b + 1, :].to_broadcast((P, N)))
                ebc = rp.tile([P, N], f32, tag="ebc")
                nc.sync.dma_start(ebc[:], e_dram[b : b + 1, :].to_broadcast((P, N)))

                nrecip4 = rp.tile([P, NCHUNK], f32, tag="nrecip4")
                e_ms = []
                for c in range(NCHUNK):
                    cc = b * NCHUNK + c
                    e_m = bp.tile([P, N], f32, tag=f"e_m{c}")
                    e_ms.append(e_m)
                    # e_m = (T_j >= T_i) * e_j ; S = rowsum(e_m)
                    nc.vector.scalar_tensor_tensor(
                        out=e_m[:], in0=tjb[:], scalar=tjc[:, cc : cc + 1], in1=ebc[:],
                        op0=ALU.is_ge, op1=ALU.mult, accum_out=s_all[:, cc : cc + 1],
                    )
                nc.vector.reciprocal(nrecip4[:], s_all[:, sl])
                nc.vector.tensor_scalar_mul(nrecip4[:], nrecip4[:], -NUDGE)
                # pn = -p'_ii (diagonal), for the batched Ln at the end
                nc.vector.tensor_mul(pn_all[:, sl], e_colall[:, sl], nrecip4[:])
                for c in range(NCHUNK):
                    cc = b * NCHUNK + c
                    e_m = e_ms[c]
                    l = bp.tile([P, N], f32, tag=f"l{c}")
                    # l = Ln(1 + EPS - e_m / S)
                    nc.scalar.activation(
                        l[:], e_m[:], ACTF.Ln, bias=b1eps[:], scale=nrecip4[:, c : c + 1]
                    )
                    scr = bp.tile([P, N], f32, tag=f"scr{c}")
                    # lsum = rowsum over the risk set of l
                    nc.vector.scalar_tensor_tensor(
                        out=scr[:], in0=e_m[:], scalar=0.0, in1=l[:],
                        op0=ALU.is_gt, op1=ALU.mult, accum_out=lsum_all[:, cc : cc + 1],
                    )

            # Batched epilogue
            logs_all = cp.tile([P, NC4], f32)
            nc.scalar.activation(logs_all[:], s_all[:], ACTF.Ln, bias=0.0, scale=1.0)
            lii_all = cp.tile([P, NC4], f32)
            nc.scalar.activation(lii_all[:], pn_all[:], ACTF.Ln, bias=b1eps[:], scale=1.0)
            d1 = cp.tile([P, NC4], f32)
            nc.vector.tensor_sub(d1[:], logs_all[:], predcm[:])
            d2 = cp.tile([P, NC4], f32)
            nc.vector.tensor_sub(d2[:], lii_all[:], lsum_all[:])
            d3 = cp.tile([P, NC4], f32)
            nc.vector.tensor_add(d3[:], d1[:], d2[:])
            c4 = cp.tile([P, NC4], f32)
            nc.vector.tensor_mul(c4[:], d3[:], iselc[:])
            acc = cp.tile([P, 1], f32)
            nc.vector.reduce_sum(acc[:], c4[:], axis=mybir.AxisListType.X)
            nc.sync.dma_start(ACC[:], acc[:])

    nc.compile()
    return nc


def _get_program():
    if "nc" not in _CACHE:
        _CACHE["nc"] = _build_program()
    return _CACHE["nc"]


def _prep_inputs(pred, target, valid_mask):
    pred = np.ascontiguousarray(pred, dtype=np.float32)
    target = np.ascontiguousarray(target, dtype=np.float32)
    valid = np.ascontiguousarray(valid_mask).astype(bool)

    tj = np.where(valid, target, np.float32(-2.0)).astype(np.float32)
    m = pred.max(axis=1, keepdims=True)  # (B,1)
    predm = (pred - m).astype(np.float32)
    tm = np.where(valid, target, np.float32(-1.0)).astype(np.float32)
    bmax = tm.max(axis=1, keepdims=True)
    is_elim = (tm < bmax) & (tm > 0) & valid
    vbm = (valid.sum(axis=1) >= 2).astype(np.float32)  # (B,)
    isel = is_elim.astype(np.float32) * vbm[:, None]
    num_valid = max(float(vbm.sum()), 1.0)

    in_maps = []
    for s in range(NCORES):
        rs = slice(s * ROWS, (s + 1) * ROWS)
        # column layouts: C[p, b*NCHUNK + c] = X[b, c*128 + p]
        def colize(x):
            return np.ascontiguousarray(
                x.reshape(ROWS, NCHUNK, P).transpose(2, 0, 1).reshape(P, NC4)
            )
        in_maps.append({
            "PREDM": predm[rs],
            "TJ": tj[rs],
            "PREDCM": colize(predm[rs]),
            "TJC": colize(tj[rs]),
            "ISELC": colize(isel[rs]),
        })
    return in_maps, num_valid


def _run(inputs, trace=False, **kwargs):
    _ensure_paths()
    from concourse.bass_utils import run_bass_kernel_spmd

    nc = _get_program()
    in_maps, num_valid = _prep_inputs(**inputs)
    res = run_bass_kernel_spmd(nc, in_maps, core_ids=list(range(NCORES)), trace=trace, **kwargs)
    total = np.float32(0.0)
    for r in res.results:
        total += np.float32(r["ACC"].sum(dtype=np.float32))
    out = np.float32(total / np.float32(num_valid))
    return np.asarray(out, dtype=np.float32), res


def kernel(pred, target, valid_mask):
    out, _ = _run({"pred": pred, "target": target, "valid_mask": valid_mask})
    return out


# revision 10
# speedup vs baseline: 1.8117x; 1.2543x over previous
"""Cox hazard loss kernel for Trainium2 (8 NeuronCores, data-parallel over batch).

Math (per batch row b, N players):
  T = where(valid, target, -2)            # -2 fill makes (T_j >= T_i) == risk_set_mask directly
  m = max_j pred[b, j]                    # i-independent logsumexp shift (folded host-side)
  e_j = exp(pred_j - m)
  mask_ij = (T_j >= T_i)
  e_m[i,j] = mask_ij * e_j ;  S_i = sum_j e_m[i,j]
  p_ij = e_m[i,j] / S_i                   # softmax over risk set
  l_ij = log(1 + EPS - p_ij)
  loss_i = is_elim_i * (log(S_i) - (pred_i - m) - sum_{j in mask} l_ij + l_ii)
  total = sum_{b,i} loss_i * valid_batch_b ; result = total / max(sum_b valid_batch_b, 1)

Per core: 16 batch rows; per row 4 chunks of 128 i's on partitions, 512 j's on free dim.
Big ops per chunk: 1 STT (mask*e + rowsum S), 1 ACT Ln, 1 STT (masked l rowsum); all SBUF.
Row broadcasts (T_j row, e row) are done by DMA with a partition-step-0 source AP.
All Exp ops batched up front and per-row epilogues batched at the end so the
scalar engine loads each activation table once (table loads cost ~1.3us each).
"""

import os
import sys

import numpy as np

B, N = 128, 512
NCORES = 8
ROWS = B // NCORES  # 16
P = 128
NCHUNK = N // P  # 4
NC4 = ROWS * NCHUNK  # 64
EPS = 1e-7
# Nudge keeps p = e*recip(S) strictly below 1 even if reciprocal rounds up,
# so Ln(1+EPS-p) never sees a non-positive argument (singleton risk sets hit p==1).
NUDGE = 1.0 - 1e-6

_CACHE = {}


def _ensure_paths():
    for p_ in ("/opt/trn_rl_repo", "/root/.axon_site/_ro/trn_rl_repo"):
        if os.path.isdir(p_) and p_ not in sys.path:
            sys.path.append(p_)


def _build_program():
    _ensure_paths()
    import concourse.bacc as bacc
    import concourse.mybir as mybir
    import concourse.tile as tile

    f32 = mybir.dt.float32
    ALU = mybir.AluOpType
    ACTF = mybir.ActivationFunctionType

    nc = bacc.Bacc("TRN2", target_bir_lowering=False, debug=False, num_devices=NCORES)

    # PREDM: pred - m (rows);  PREDCM: same, column-layout;  TJ: masked target rows;
    # TJC: column-layout;  ISELC: is_elim * valid_batch, column-layout.
    PREDM = nc.dram_tensor("PREDM", (ROWS, N), f32, kind="ExternalInput").ap()
    TJ = nc.dram_tensor("TJ", (ROWS, N), f32, kind="ExternalInput").ap()
    PREDCM = nc.dram_tensor("PREDCM", (P, NC4), f32, kind="ExternalInput").ap()
    TJC = nc.dram_tensor("TJC", (P, NC4), f32, kind="ExternalInput").ap()
    ISELC = nc.dram_tensor("ISELC", (P, NC4), f32, kind="ExternalInput").ap()
    ACC = nc.dram_tensor("ACC", (P, 1), f32, kind="ExternalOutput").ap()

    with tile.TileContext(nc) as tc:
        with (
            tc.tile_pool(name="const", bufs=1) as cp,
            tc.tile_pool(name="row", bufs=3) as rp,
            tc.tile_pool(name="big", bufs=2) as bp,
            tc.tile_pool(name="dram", bufs=1, space="DRAM") as dp,
        ):
            predcm = cp.tile([P, NC4], f32)
            nc.sync.dma_start(predcm[:], PREDCM[:])
            tjc = cp.tile([P, NC4], f32)
            nc.sync.dma_start(tjc[:], TJC[:])
            iselc = cp.tile([P, NC4], f32)
            nc.sync.dma_start(iselc[:], ISELC[:])
            predm_all = cp.tile([ROWS, N], f32)
            nc.sync.dma_start(predm_all[:], PREDM[:])

            # Batched Exps (one table load)
            e_all = cp.tile([ROWS, N], f32)
            nc.scalar.activation(e_all[:], predm_all[:], ACTF.Exp, bias=0.0, scale=1.0)
            # Bounce e rows through DRAM so they can be partition-broadcast by DMA
            # (SBUF source APs cannot have a zero partition step).
            e_dram = dp.tile([ROWS, N], f32)
            nc.sync.dma_start(e_dram[:], e_all[:])
            e_colall = cp.tile([P, NC4], f32)
            nc.scalar.activation(e_colall[:], predcm[:], ACTF.Exp, bias=0.0, scale=1.0)

            # Full-run accumulators, one column per (row, chunk)
            s_all = cp.tile([P, NC4], f32)
            lsum_all = cp.tile([P, NC4], f32)
            pn_all = cp.tile([P, NC4], f32)

            for b in range(ROWS):
                sl = slice(b * NCHUNK, (b + 1) * NCHUNK)
                # Broadcast T row (from DRAM) and e row (from SBUF) across partitions.
                tjb = rp.tile([P, N], f32, tag="tjb")
                nc.sync.dma_start(tjb[:], TJ[b : b + 1, :].to_broadcast((P, N)))
                ebc = rp.tile([P, N], f32, tag="ebc")
                nc.sync.dma_start(ebc[:], e_dram[b : b + 1, :].to_broadcast((P, N)))

                nrecip4 = rp.tile([P, NCHUNK], f32, tag="nrecip4")
                e_ms = []
                for c in range(NCHUNK):
                    cc = b * NCHUNK + c
                    e_m = bp.tile([P, N], f32, tag=f"e_m{c}")
                    e_ms.append(e_m)
                    # e_m = (T_j >= T_i) * e_j ; S = rowsum(e_m)
                    nc.vector.scalar_tensor_tensor(
                        out=e_m[:], in0=tjb[:], scalar=tjc[:, cc : cc + 1], in1=ebc[:],
                        op0=ALU.is_ge, op1=ALU.mult, accum_out=s_all[:, cc : cc + 1],
                    )
                nc.vector.reciprocal(nrecip4[:], s_all[:, sl])
                nc.vector.tensor_scalar_mul(nrecip4[:], nrecip4[:], -NUDGE)
                # pn = -p'_ii (diagonal), for the batched Ln at the end
                nc.vector.tensor_mul(pn_all[:, sl], e_colall[:, sl], nrecip4[:])
                for c in range(NCHUNK):
                    cc = b * NCHUNK + c
                    e_m = e_ms[c]
                    l = bp.tile([P, N], f32, tag=f"l{c}")
                    # l = Ln(1 - e_m / S); unmasked entries hit Ln(1.0) == 0 exactly,
                    # so the ACT-side accumulator IS the masked row sum.
                    nc.scalar.activation(
                        l[:], e_m[:], ACTF.Ln, bias=1.0, scale=nrecip4[:, c : c + 1],
                        accum_out=lsum_all[:, cc : cc + 1],
                    )

            # Batched epilogue
            logs_all = cp.tile([P, NC4], f32)
            nc.scalar.activation(logs_all[:], s_all[:], ACTF.Ln, bias=0.0, scale=1.0)
            # Same bias as the bulk path so the diagonal exclusion cancels exactly.
            lii_all = cp.tile([P, NC4], f32)
            nc.scalar.activation(lii_all[:], pn_all[:], ACTF.Ln, bias=1.0, scale=1.0)
            d1 = cp.tile([P, NC4], f32)
            nc.vector.tensor_sub(d1[:], logs_all[:], predcm[:])
            d2 = cp.tile([P, NC4], f32)
            nc.vector.tensor_sub(d2[:], lii_all[:], lsum_all[:])
            d3 = cp.tile([P, NC4], f32)
            nc.vector.tensor_add(d3[:], d1[:], d2[:])
            c4 = cp.tile([P, NC4], f32)
            nc.vector.tensor_mul(c4[:], d3[:], iselc[:])
            acc = cp.tile([P, 1], f32)
            nc.vector.reduce_sum(acc[:], c4[:], axis=mybir.AxisListType.X)
            nc.sync.dma_start(ACC[:], acc[:])

    nc.compile()
    return nc


def _get_program():
    if "nc" not in _CACHE:
        _CACHE["nc"] = _build_program()
    return _CACHE["nc"]


def _prep_inputs(pred, target, valid_mask):
    pred = np.ascontiguousarray(pred, dtype=np.float32)
    target = np.ascontiguousarray(target, dtype=np.float32)
    valid = np.ascontiguousarray(valid_mask).astype(bool)

    tj = np.where(valid, target, np.float32(-2.0)).astype(np.float32)
    m = pred.max(axis=1, keepdims=True)  # (B,1)
    predm = (pred - m).astype(np.float32)
    tm = np.where(valid, target, np.float32(-1.0)).astype(np.float32)
    bmax = tm.max(axis=1, keepdims=True)
    is_elim = (tm < bmax) & (tm > 0) & valid
    vbm = (valid.sum(axis=1) >= 2).astype(np.float32)  # (B,)
    isel = is_elim.astype(np.float32) * vbm[:, None]
    num_valid = max(float(vbm.sum()), 1.0)

    in_maps = []
    for s in range(NCORES):
        rs = slice(s * ROWS, (s + 1) * ROWS)
        # column layouts: C[p, b*NCHUNK + c] = X[b, c*128 + p]
        def colize(x):
            return np.ascontiguousarray(
                x.reshape(ROWS, NCHUNK, P).transpose(2, 0, 1).reshape(P, NC4)
            )
        in_maps.append({
            "PREDM": predm[rs],
            "TJ": tj[rs],
            "PREDCM": colize(predm[rs]),
            "TJC": colize(tj[rs]),
            "ISELC": colize(isel[rs]),
        })
    return in_maps, num_valid


def _run(inputs, trace=False, **kwargs):
    _ensure_paths()
    from concourse.bass_utils import run_bass_kernel_spmd

    nc = _get_program()
    in_maps, num_valid = _prep_inputs(**inputs)
    res = run_bass_kernel_spmd(nc, in_maps, core_ids=list(range(NCORES)), trace=trace, **kwargs)
    total = np.float32(0.0)
    for r in res.results:
        total += np.float32(r["ACC"].sum(dtype=np.float32))
    out = np.float32(total / np.float32(num_valid))
    return np.asarray(out, dtype=np.float32), res


def kernel(pred, target, valid_mask):
    out, _ = _run({"pred": pred, "target": target, "valid_mask": valid_mask})
    return out


# revision 12
# speedup vs baseline: 2.0496x; 1.1313x over previous
"""Cox hazard loss kernel for Trainium2 (8 NeuronCores, data-parallel over batch).

Math (per batch row b, N players):
  T = where(valid, target, -2)            # -2 fill makes (T_j >= T_i) == risk_set_mask directly
  m = max_j pred[b, j]                    # i-independent logsumexp shift (folded host-side)
  e_j = exp(pred_j - m)
  mask_ij = (T_j >= T_i)
  e_m[i,j] = mask_ij * e_j ;  S_i = sum_j e_m[i,j]
  p_ij = e_m[i,j] / S_i                   # softmax over risk set
  l_ij = log(1 + EPS - p_ij)
  loss_i = is_elim_i * (log(S_i) - (pred_i - m) - sum_{j in mask} l_ij + l_ii)
  total = sum_{b,i} loss_i * valid_batch_b ; result = total / max(sum_b valid_batch_b, 1)

Per core: 16 batch rows; per row 4 chunks of 128 i's on partitions, 512 j's on free dim.
Big ops per chunk: 1 STT (mask*e + rowsum S), 1 ACT Ln, 1 STT (masked l rowsum); all SBUF.
Row broadcasts (T_j row, e row) are done by DMA with a partition-step-0 source AP.
All Exp ops batched up front and per-row epilogues batched at the end so the
scalar engine loads each activation table once (table loads cost ~1.3us each).
"""

import os
import sys

import numpy as np

B, N = 128, 512
NCORES = 8
ROWS = B // NCORES  # 16
P = 128
NCHUNK = N // P  # 4
NC4 = ROWS * NCHUNK  # 64
EPS = 1e-7
# Nudge keeps p = e*recip(S) strictly below 1 even if reciprocal rounds up,
# so Ln(1+EPS-p) never sees a non-positive argument (singleton risk sets hit p==1).
NUDGE = 1.0 - 1e-6

_CACHE = {}


def _ensure_paths():
    for p_ in ("/opt/trn_rl_repo", "/root/.axon_site/_ro/trn_rl_repo"):
        if os.path.isdir(p_) and p_ not in sys.path:
            sys.path.append(p_)


def _build_program():
    _ensure_paths()
    import concourse.bacc as bacc
    import concourse.mybir as mybir
    import concourse.tile as tile

    f32 = mybir.dt.float32
    ALU = mybir.AluOpType
    ACTF = mybir.ActivationFunctionType

    nc = bacc.Bacc("TRN2", target_bir_lowering=False, debug=False, num_devices=NCORES)

    # PREDM: pred - m (rows);  PREDCM: same, column-layout;  TJ: masked target rows;
    # TJC: column-layout;  ISELC: is_elim * valid_batch, column-layout.
    PREDM = nc.dram_tensor("PREDM", (ROWS, N), f32, kind="ExternalInput").ap()
    TJ = nc.dram_tensor("TJ", (ROWS, N), f32, kind="ExternalInput").ap()
    PREDCM = nc.dram_tensor("PREDCM", (P, NC4), f32, kind="ExternalInput").ap()
    TJC = nc.dram_tensor("TJC", (P, NC4), f32, kind="ExternalInput").ap()
    ISELC = nc.dram_tensor("ISELC", (P, NC4), f32, kind="ExternalInput").ap()
    ACC = nc.dram_tensor("ACC", (P, 1), f32, kind="ExternalOutput").ap()

    with tile.TileContext(nc) as tc:
        with (
            tc.tile_pool(name="const", bufs=1) as cp,
            tc.tile_pool(name="row", bufs=3) as rp,
            tc.tile_pool(name="big", bufs=2) as bp,
            tc.tile_pool(name="dram", bufs=1, space="DRAM") as dp,
        ):
            predcm = cp.tile([P, NC4], f32)
            nc.sync.dma_start(predcm[:], PREDCM[:])
            tjc = cp.tile([P, NC4], f32)
            nc.sync.dma_start(tjc[:], TJC[:])
            iselc = cp.tile([P, NC4], f32)
            nc.sync.dma_start(iselc[:], ISELC[:])
            predm_all = cp.tile([ROWS, N], f32)
            nc.sync.dma_start(predm_all[:], PREDM[:])

            # Batched Exps (one table load)
            e_all = cp.tile([ROWS, N], f32)
            nc.scalar.activation(e_all[:], predm_all[:], ACTF.Exp, bias=0.0, scale=1.0)
            # Bounce e rows through DRAM so they can be partition-broadcast by DMA
            # (SBUF source APs cannot have a zero partition step).
            e_dram = dp.tile([ROWS, N], f32)
            nc.sync.dma_start(e_dram[:], e_all[:])
            e_colall = cp.tile([P, NC4], f32)
            nc.scalar.activation(e_colall[:], predcm[:], ACTF.Exp, bias=0.0, scale=1.0)

            # Full-run accumulators, one column per (row, chunk)
            s_all = cp.tile([P, NC4], f32)
            lsum_all = cp.tile([P, NC4], f32)
            pn_all = cp.tile([P, NC4], f32)

            for b in range(ROWS):
                sl = slice(b * NCHUNK, (b + 1) * NCHUNK)
                # Broadcast T row (from DRAM) and e row (from SBUF) across partitions.
                tjb = rp.tile([P, N], f32, tag="tjb")
                nc.sync.dma_start(tjb[:], TJ[b : b + 1, :].to_broadcast((P, N)))
                ebc = rp.tile([P, N], f32, tag="ebc")
                nc.sync.dma_start(ebc[:], e_dram[b : b + 1, :].to_broadcast((P, N)))

                nrecip4 = rp.tile([P, NCHUNK], f32, tag="nrecip4")
                e_ms = []
                for c in range(NCHUNK):
                    cc = b * NCHUNK + c
                    # Rows are sorted by T ascending, so the risk set of any i in
                    # chunk c lives in columns [128c, 512) — shrink the op width.
                    w = N - c * P
                    e_m = bp.tile([P, w], f32, tag=f"e_m{c}")
                    e_ms.append(e_m)
                    # e_m = (T_j >= T_i) * e_j ; S = rowsum(e_m)
                    nc.vector.scalar_tensor_tensor(
                        out=e_m[:], in0=tjb[:, c * P :], scalar=tjc[:, cc : cc + 1],
                        in1=ebc[:, c * P :],
                        op0=ALU.is_ge, op1=ALU.mult, accum_out=s_all[:, cc : cc + 1],
                    )
                nc.vector.reciprocal(nrecip4[:], s_all[:, sl])
                nc.vector.tensor_scalar_mul(nrecip4[:], nrecip4[:], -NUDGE)
                # pn = -p'_ii (diagonal), for the batched Ln at the end
                nc.vector.tensor_mul(pn_all[:, sl], e_colall[:, sl], nrecip4[:])
                for c in range(NCHUNK):
                    cc = b * NCHUNK + c
                    w = N - c * P
                    e_m = e_ms[c]
                    l = bp.tile([P, w], f32, tag=f"l{c}")
                    # l = Ln(1 - e_m / S); unmasked entries hit Ln(1.0) == 0 exactly,
                    # so the ACT-side accumulator IS the masked row sum.
                    nc.scalar.activation(
                        l[:], e_m[:], ACTF.Ln, bias=1.0, scale=nrecip4[:, c : c + 1],
                        accum_out=lsum_all[:, cc : cc + 1],
                    )

            # Batched epilogue
            logs_all = cp.tile([P, NC4], f32)
            nc.scalar.activation(logs_all[:], s_all[:], ACTF.Ln, bias=0.0, scale=1.0)
            # Same bias as the bulk path so the diagonal exclusion cancels exactly.
            lii_all = cp.tile([P, NC4], f32)
            nc.scalar.activation(lii_all[:], pn_all[:], ACTF.Ln, bias=1.0, scale=1.0)
            d1 = cp.tile([P, NC4], f32)
            nc.vector.tensor_sub(d1[:], logs_all[:], predcm[:])
            d2 = cp.tile([P, NC4], f32)
            nc.vector.tensor_sub(d2[:], lii_all[:], lsum_all[:])
            d3 = cp.tile([P, NC4], f32)
            nc.vector.tensor_add(d3[:], d1[:], d2[:])
            c4 = cp.tile([P, NC4], f32)
            nc.vector.tensor_mul(c4[:], d3[:], iselc[:])
            acc = cp.tile([P, 1], f32)
            nc.vector.reduce_sum(acc[:], c4[:], axis=mybir.AxisListType.X)
            nc.sync.dma_start(ACC[:], acc[:])

    nc.compile()
    return nc


def _get_program():
    if "nc" not in _CACHE:
        _CACHE["nc"] = _build_program()
    return _CACHE["nc"]


def _prep_inputs(pred, target, valid_mask):
    pred = np.ascontiguousarray(pred, dtype=np.float32)
    target = np.ascontiguousarray(target, dtype=np.float32)
    valid = np.ascontiguousarray(valid_mask).astype(bool)

    tj = np.where(valid, target, np.float32(-2.0)).astype(np.float32)
    m = pred.max(axis=1, keepdims=True)  # (B,1)
    predm = (pred - m).astype(np.float32)
    tm = np.where(valid, target, np.float32(-1.0)).astype(np.float32)
    bmax = tm.max(axis=1, keepdims=True)
    is_elim = (tm < bmax) & (tm > 0) & valid
    vbm = (valid.sum(axis=1) >= 2).astype(np.float32)  # (B,)
    isel = is_elim.astype(np.float32) * vbm[:, None]
    num_valid = max(float(vbm.sum()), 1.0)

    # Sort each row by T ascending so risk sets become rank-suffixes; the kernel
    # then only touches columns [128c, 512) for i-chunk c. The loss sums over i,
    # so no un-permutation is needed.
    order = np.argsort(tj, axis=1, kind="stable")
    tj = np.take_along_axis(tj, order, axis=1)
    predm = np.take_along_axis(predm, order, axis=1)
    isel = np.take_along_axis(isel, order, axis=1)

    in_maps = []
    for s in range(NCORES):
        rs = slice(s * ROWS, (s + 1) * ROWS)
        # column layouts: C[p, b*NCHUNK + c] = X[b, c*128 + p]
        def colize(x):
            return np.ascontiguousarray(
                x.reshape(ROWS, NCHUNK, P).transpose(2, 0, 1).reshape(P, NC4)
            )
        in_maps.append({
            "PREDM": predm[rs],
            "TJ": tj[rs],
            "PREDCM": colize(predm[rs]),
            "TJC": colize(tj[rs]),
            "ISELC": colize(isel[rs]),
        })
    return in_maps, num_valid


def _run(inputs, trace=False, **kwargs):
    _ensure_paths()
    from concourse.bass_utils import run_bass_kernel_spmd

    nc = _get_program()
    in_maps, num_valid = _prep_inputs(**inputs)
    res = run_bass_kernel_spmd(nc, in_maps, core_ids=list(range(NCORES)), trace=trace, **kwargs)
    total = np.float32(0.0)
    for r in res.results:
        total += np.float32(r["ACC"].sum(dtype=np.float32))
    out = np.float32(total / np.float32(num_valid))
    return np.asarray(out, dtype=np.float32), res


def kernel(pred, target, valid_mask):
    out, _ = _run({"pred": pred, "target": target, "valid_mask": valid_mask})
    return out
